# revision 26
# baseline (speedup 1.0000x reference)
"""Trainium2 Bass kernel for nn_AttentionalGNN (self-contained).

  xs/xt = standardize(p_src/p_tar).T ; ds/dt = mlp_dis(standardize(dis).T)
  delta0 = attn(xs, xt, xt); delta1 = attn(xt, xs, xs)
  ps = delta0*xt; pt = delta1*xs
  delta0' = attn(ds, dt, ps); delta1' = attn(dt, ds, pt)
  out_s = xs + mlp(cat(xs, delta0')); out_t likewise
  return ||mean_n(out_s) - mean_n(out_t)||^2

8-core SPMD: scale stats replicated; queries sharded 512/core for all four
attention calls (keys/values replicated); mlp_dis sharded over N with BN-stat
AllReduce + AllGather; the two round-1 deltas are AllGathered separately
(delta0 right after round 1a so the wire time hides under round 1b; delta1
after 1b, hiding under round 2a); final MLP sharded over N with AllReduced BN
stats.

Since standardize() gives every feature column exactly zero mean, the
residual xs/xt terms vanish from the final MMD: the scalar reduces to
||m2 @ (mean_n relu_s - mean_n relu_t)||^2, so m2 is applied to the
512-vector of relu means (fp32) instead of the full N columns.

Attention uses transposed scores (keys on partitions, queries on free) so no
transposes are needed anywhere: scoresT = K_h^T Q_h via one K=64 matmul per
key m-tile; exp on ScalarE (scale=1/8, no max subtraction - scores are O(10));
softmax denominator comes from a ones column prepended to V^T inside the PV
matmul; per-head normalization happens post-loop with a PE-broadcast
reciprocal. K-projection bias is dropped entirely: it adds a per-query
constant to every key's score, which softmax cancels. V bias is folded into
the merge bias (bm_eff = am_b + am_w @ av_b). Head channels are permuted
host-side (d*4+h -> h*64+d) so head slices are contiguous.

All bf16 weights ship as one packed [128, 5376] blob (single DMA on the ACT
HWDGE queue so it never queues behind the big input loads on SP), biases as
one [128, 24] fp32 blob. 1/sqrt is computed on VectorE with the bit-trick +
2 Newton steps so ScalarE never swaps activation tables away from exp.
"""

import numpy as np
import ml_dtypes

D, H, HD, S, N, EPS = 256, 4, 64, 128, 4096, 1e-5
NC = 8
NQ = N // NC            # 512 queries per core
MT = N // 128           # 32 key m-tiles
HB = HD + 1             # per-head V^T block: [ones | V] = 65 cols

# bf16 weight blob column offsets
WQ0, WK0, WV0, WD2, WD1, WM1, WWM = 0, 512, 1024, 1536, 2048, 2304, 4352
NWB = 5376
# fp32 bias blob columns: bq0:2 bm2:4 d1b4:6 d1g6:8 d1be8:10 d2b10:12
# m1b12:16 m1g16:20 m1be20:24
NBB = 24

_CACHE = {}


def _build_program(single=False):
    """single=True: replace collectives with same-size local DMA copies so the
    program is single-core simulatable - timing proxy only."""
    import contextlib
    import concourse.bass as bass
    import concourse.bacc as bacc
    import concourse.tile as tile
    import concourse.mybir as mybir

    FP32 = mybir.dt.float32
    BF16 = mybir.dt.bfloat16
    I32 = mybir.dt.int32
    AF = mybir.ActivationFunctionType
    ALU = mybir.AluOpType
    AX = mybir.AxisListType

    nc = bacc.Bacc(
        "TRN2",
        target_bir_lowering=False,
        debug=False,
        enable_asserts=False,
        num_devices=NC,
    )

    def din(name, shape, dt):
        return nc.dram_tensor(name, shape, dt, kind="ExternalInput").ap()

    psT = din("psT", [D, N], BF16)
    ptT = din("ptT", [D, N], BF16)
    dsT = din("dsT", [S, N], BF16)
    dtT = din("dtT", [S, N], BF16)
    ops = din("ops", [D, NQ], BF16)
    opt_ = din("opt", [D, NQ], BF16)
    ods = din("ods", [S, NQ], BF16)
    odt = din("odt", [S, NQ], BF16)
    wblob = din("wblob", [128, NWB], BF16)
    m2f = din("m2f", [128, 4 * D], FP32)
    bblob = din("bblob", [128, NBB], FP32)
    out_dram = nc.dram_tensor("out", [1, 1], FP32, kind="ExternalOutput").ap()

    RG = [list(range(NC))]

    with tile.TileContext(nc) as tc:
        st = contextlib.ExitStack()
        PA = st.enter_context(tc.tile_pool(name="persistA", bufs=1))
        Ppr = st.enter_context(tc.tile_pool(name="probs", bufs=4))
        Psc = st.enter_context(
            tc.tile_pool(name="psum_sc", bufs=3, space=bass.MemorySpace.PSUM))
        Pout = st.enter_context(
            tc.tile_pool(name="psum_out", bufs=2, space=bass.MemorySpace.PSUM))
        Dram = st.enter_context(tc.tile_pool(name="dram", bufs=1, space="DRAM"))

        def pa(name, shape, dt, tag=None):
            return PA.tile(shape, dt, name=name, tag=tag or name)

        # --- persistA: needed from preprocessing onward ---
        WB = pa("WB", [128, NWB], BF16)
        M2F = pa("M2F", [128, 4 * D], FP32)
        BB = pa("BB", [128, NBB], FP32)
        xs_own_bf = pa("xs_own_bf", [128, 2, 512], BF16)
        xt_own_bf = pa("xt_own_bf", [128, 2, 512], BF16)
        ds_own_bf = pa("ds_own_bf", [128, 2, 512], BF16)
        dt_own_bf = pa("dt_own_bf", [128, 2, 512], BF16)
        x1d_s = pa("x1d_s", [128, 2, 512], FP32)
        x1d_t = pa("x1d_t", [128, 2, 512], FP32)
        sc_ds = pa("sc_ds", [128, 512], BF16)
        sc_dt = pa("sc_dt", [128, 512], BF16)
        r_d = pa("r_d", [128, 2, 512], BF16)
        Qb = pa("Qb", [128, 2, 512], BF16)
        An = pa("An", [64, 4, 512], BF16)       # attn out per head (raw->normed)
        d_bf = pa("d_bf", [128, 2, 512], BF16)  # round-1 delta0 own
        d1own_bf = pa("d1own_bf", [128, 2, 512], BF16)
        dp0_bf = pa("dp0_bf", [128, 2, 512], BF16)
        dp1_bf = pa("dp1_bf", [128, 2, 512], BF16)
        ones = pa("ones", [128, 64], FP32)
        sqd = pa("sqd", [128, 2, 512], FP32)
        bns = pa("bns", [128, 8, 6], FP32)
        par = pa("par", [128, 2, 4], FP32)
        tot = pa("tot", [128, 2, 4], FP32)
        parf = pa("parf", [128, 4, 4], FP32)
        totf = pa("totf", [128, 4, 4], FP32)
        mrel = pa("mrel", [128, 4, 2], FP32)
        totr = pa("totr", [128, 4, 2], FP32)
        mdif = pa("mdif", [128, 4, 1], FP32)
        cfw = pa("cfw", [128, 8, 4], FP32)   # vectorized BN coef scratch
        dlt = pa("dlt", [128, 2, 1], FP32)
        dsq = pa("dsq", [128, 2, 1], FP32)
        res = pa("res", [1, 1], FP32)
        sv = pa("sv", [128, 200], FP32)   # scalar scratch, allocator below

        _svc = [0]

        def scol(n=1):
            b = _svc[0]
            _svc[0] += n
            assert _svc[0] <= 200
            return [sv[:, b + i:b + i + 1] for i in range(n)]

        dma = nc.sync.dma_start
        shr = "Local" if single else "Shared"

        def coll(kind, op, in_t, out_t, in_rows):
            if not single:
                nc.gpsimd.collective_compute(kind, op, replica_groups=RG,
                                             ins=[in_t.opt()],
                                             outs=[out_t.opt()])
            elif kind == "AllGather":
                for r_ in range(NC):
                    dma(out_t[r_ * in_rows:(r_ + 1) * in_rows, :], in_t[:, :])
            else:
                dma(out_t[:, :], in_t[:, :])

        # weight slice helpers (og in units of 128 output cols)
        def wsl(base, cg, og, cgw=256):
            c0 = base + cg * cgw + og * 128
            return WB[:, c0:c0 + 128]

        def bb(col):
            return BB[:, col:col + 1]

        def rsqrt_into(t, r, s1, s2, var_ap, eps):
            """1/sqrt(var+eps) -> r on VectorE: bit-trick seed + 2 Newton.
            t/r/s1/s2 are same-shape fp32 scratch APs."""
            nc.vector.tensor_scalar_add(t, var_ap, float(eps))
            ti = t.bitcast(I32)
            ri = r.bitcast(I32)
            nc.vector.tensor_scalar(ri, ti, 1, None,
                                    ALU.logical_shift_right)
            nc.vector.tensor_scalar(ri, ri, 0x5F3759DF, -1,
                                    ALU.subtract, ALU.mult)
            for _ in range(2):
                nc.vector.tensor_mul(s1, r, r)
                nc.vector.tensor_mul(s1, s1, t)
                nc.vector.tensor_scalar(s2, s1, -0.5, 1.5, ALU.mult, ALU.add)
                nc.vector.tensor_mul(r, r, s2)
            return r

        def rsqrt_dve(var_ap, eps):
            t, r, s1, s2 = scol(4)
            return rsqrt_into(t, r, s1, s2, var_ap, eps)

        # weight blobs ride the ACT HWDGE queue so the big input loads own SP
        nc.scalar.dma_start(WB[:, :], wblob[:, :])
        nc.scalar.dma_start(M2F[:, :], m2f[:, :])
        nc.scalar.dma_start(BB[:, :], bblob[:, :])
        nc.vector.memset(ones[:, :], 1.0)

        # ------------- standardize p_src/p_tar (per row-group stream) -------------
        # full tensors are streamed only for the population stats; just the
        # own 512-column shard is scaled and kept (K/V are projected sharded
        # and AllGathered post-projection)
        with tc.tile_pool(name="pin", bufs=3) as Pin:
            for nm, srcT, ownT, own_b16 in (
                ("s", psT, ops, xs_own_bf),
                ("t", ptT, opt_, xt_own_bf),
            ):
                for g in range(2):
                    pbuf = Pin.tile([128, 8, 512], BF16, tag="pbuf",
                                    name=f"pbuf_{nm}{g}")
                    obuf = Pin.tile([128, 512], BF16, tag="obuf",
                                    name=f"obuf_{nm}{g}")
                    r = slice(g * 128, (g + 1) * 128)
                    dma(pbuf[:, :, :], srcT[r, :].rearrange("p (c f) -> p c f", f=512))
                    dma(obuf[:, :], ownT[r, :])
                    for c in range(8):
                        nc.vector.bn_stats(bns[:, c, :], pbuf[:, c, :])
                    ag2 = pa(f"ag_{nm}{g}", [128, 2], FP32)
                    nc.vector.bn_aggr(ag2[:, :], bns[:, :, :])
                    inv = rsqrt_dve(ag2[:, 1:2], 0.0)
                    (nb,) = scol(1)
                    nc.vector.tensor_mul(nb, ag2[:, 0:1], inv)
                    nc.vector.tensor_scalar_mul(nb, nb, -1.0)
                    nc.scalar.activation(own_b16[:, g, :], obuf[:, :],
                                         AF.Identity, bias=nb, scale=inv)

            # ------------- dis stats + own shard scale + d1 conv -------------
            for nm, srcT, ownT, scdst in (
                ("ds", dsT, ods, sc_ds),
                ("dt", dtT, odt, sc_dt),
            ):
                dbuf = Pin.tile([128, 8, 512], BF16, tag="pbuf", name=f"dbuf_{nm}")
                obuf = Pin.tile([128, 512], BF16, tag="obuf", name=f"obuf_{nm}")
                dma(dbuf[:, :, :], srcT[:, :].rearrange("p (c f) -> p c f", f=512))
                dma(obuf[:, :], ownT[:, :])
                for c in range(8):
                    nc.vector.bn_stats(bns[:, c, :], dbuf[:, c, :])
                ag2 = pa(f"ag_{nm}", [128, 2], FP32)
                nc.vector.bn_aggr(ag2[:, :], bns[:, :, :])
                inv = rsqrt_dve(ag2[:, 1:2], 0.0)
                (nb,) = scol(1)
                nc.vector.tensor_mul(nb, ag2[:, 0:1], inv)
                nc.vector.tensor_scalar_mul(nb, nb, -1.0)
                nc.scalar.activation(scdst[:, :], obuf[:, :],
                                     AF.Identity, bias=nb, scale=inv)

        for src, dst in ((sc_ds, x1d_s), (sc_dt, x1d_t)):
            for og in range(2):
                mp = Psc.tile([128, 2, 512], FP32, tag="sc", name="mp_d1")
                nc.tensor.matmul(mp[:, 0, :], WB[:, WD1 + og * 128:WD1 + (og + 1) * 128],
                                 src[:, :], start=True, stop=True)
                nc.vector.tensor_scalar_add(dst[:, og, :], mp[:, 0, :],
                                            bb(4 + og))
        # partial BN stats for both d1 outputs -> one AllReduce
        for i, x1 in enumerate((x1d_s, x1d_t)):
            nc.vector.reduce_sum(par[:, :, 2 * i], x1[:, :, :], axis=AX.X)
            nc.vector.tensor_mul(sqd[:, :, :], x1[:, :, :], x1[:, :, :])
            nc.vector.reduce_sum(par[:, :, 2 * i + 1], sqd[:, :, :], axis=AX.X)
        ar_in = Dram.tile([D, 4], FP32, name="ar_in")
        ar_out = Dram.tile([D, 4], FP32, name="ar_out", addr_space=shr)
        dma(ar_in[:, :].rearrange("(g p) c -> p g c", g=2), par[:, :, :])
        # NOTE: the AllReduce itself is launched after the round-1 K/V
        # AllGathers (below) - the CC core runs collectives strictly in
        # order and this one isn't consumed until after round 1a

        # --- persistB: attention-era tensors ---
        PB = st.enter_context(tc.tile_pool(name="persistB", bufs=1))

        VT = PB.tile([128, MT, H * HB], BF16, name="VT", tag="VT")
        Kb = PB.tile([128, 2, 8, 512], BF16, name="Kb", tag="Kb")
        VTo = PB.tile([128, 4, H * HB], BF16, name="VTo", tag="VTo")
        KbO = PB.tile([128, 2, 512], BF16, name="KbO", tag="KbO")
        for h in range(H):
            nc.vector.memset(VTo[:, :, h * HB + HD], 1.0)

        # K/V projected on the own 512-key shard only, then AllGathered
        # (post-projection wire = pre-projection, but 8x fewer PE matmuls);
        # the ones columns ride through the AllGather from VTo.
        def kv_own(tag, k_src, v_src):
            for og in range(2):
                kp = Psc.tile([128, 2, 512], FP32, tag="sc", name=f"kp_{tag}")
                for cg in range(2):
                    nc.tensor.matmul(kp[:, 0, :], wsl(WK0, cg, og),
                                     k_src[:, cg, :],
                                     start=(cg == 0), stop=(cg == 1))
                nc.vector.tensor_copy(KbO[:, og, :], kp[:, 0, :])
            for m in range(4):
                cg0, f0 = divmod(m * 128, 512)
                vp = Psc.tile([128, 2, 512], FP32, tag="sc", name=f"vp_{tag}")
                for cg in range(2):
                    nc.tensor.matmul(vp[:, 0, 0:256],
                                     v_src[:, cg, f0:f0 + 128],
                                     WB[:, WV0 + cg * 256:WV0 + (cg + 1) * 256],
                                     start=(cg == 0), stop=(cg == 1))
                nc.vector.tensor_copy(
                    VTo[:, m, :].rearrange("p (h c) -> p h c", c=HB)[:, :, 0:HD],
                    vp[:, 0, 0:256].rearrange("p (h c) -> p h c", c=HD))
            agk_in = Dram.tile([D, NQ], BF16, name=f"agk_in_{tag}")
            agk_out = Dram.tile([NC * D, NQ], BF16, name=f"agk_out_{tag}",
                                addr_space=shr)
            agv_in = Dram.tile([NQ, H * HB], BF16, name=f"agv_in_{tag}")
            agv_out = Dram.tile([N, H * HB], BF16, name=f"agv_out_{tag}",
                                addr_space=shr)
            dma(agk_in[:, :].rearrange("(g p) f -> p g f", g=2), KbO[:, :, :])
            dma(agv_in[:, :].rearrange("(m p) c -> p m c", m=4), VTo[:, :, :])
            coll("AllGather", ALU.bypass, agk_in, agk_out, D)
            coll("AllGather", ALU.bypass, agv_in, agv_out, NQ)
            return agk_out, agv_out

        def attn_core(tag, ags, q_own_bf, merge_b16):
            agk_out, agv_out = ags
            # gather the AllGathered K/V (waits on the collectives; WAR on
            # Kb/VT against the previous round's reads is sem-handled)
            agkv = agk_out[:, :].rearrange("(r g p) f -> p g r f", g=2, p=128)
            for g in range(2):
                dma(Kb[:, g, :, :], agkv[:, g, :, :])
            agvv = agv_out[:, :].rearrange("(r m p) c -> m p r c", m=4, p=128)
            vtv = VT[:, :, :].rearrange("p (r m) c -> m p r c", m=4)
            for m in range(4):
                dma(vtv[m], agvv[m])
            # Q projection (+bias)
            qp = Psc.tile([128, 2, 512], FP32, tag="sc", name=f"qp_{tag}")
            for og in range(2):
                for cg in range(2):
                    nc.tensor.matmul(qp[:, og, :], wsl(WQ0, cg, og),
                                     q_own_bf[:, cg, :],
                                     start=(cg == 0), stop=(cg == 1))
            for og in range(2):
                nc.scalar.activation(Qb[:, og, :], qp[:, og, :], AF.Identity,
                                     bias=bb(0 + og))
            # streaming attention per head, PV pipelined one group behind
            recs = []
            for h in range(H):
                hg, hp = h // 2, (h % 2) * 64
                op = Pout.tile([65, 512], FP32, tag="out", name=f"op_{tag}{h}")
                prev = None
                for g in range(16):
                    sc = Psc.tile([128, 2, 512], FP32, tag="sc", name=f"sc_{tag}")
                    for j in range(2):
                        m = g * 2 + j
                        c, f0 = divmod(m * 128, 512)
                        nc.tensor.matmul(sc[:, j, :],
                                         Kb[hp:hp + 64, hg, c, f0:f0 + 128],
                                         Qb[hp:hp + 64, hg, :],
                                         start=True, stop=True)
                    pr = Ppr.tile([128, 2, 512], BF16, tag="pr", name=f"pr_{tag}",
                                  bufs=3)
                    nc.scalar.activation(pr[:, :, :], sc[:, :, :], AF.Exp,
                                         scale=0.125)
                    if prev is not None:
                        for j in range(2):
                            m = prev[0] * 2 + j
                            nc.tensor.matmul(
                                op[:, :], VT[:, m, h * HB:(h + 1) * HB],
                                prev[1][:, j, :], start=(m == 0), stop=False)
                    prev = (g, pr)
                for j in range(2):
                    m = prev[0] * 2 + j
                    nc.tensor.matmul(op[:, :], VT[:, m, h * HB:(h + 1) * HB],
                                     prev[1][:, j, :], start=False,
                                     stop=(m == MT - 1))
                # evacuate raw numerator (bf16); denominator: psum row 64 ->
                # sbuf row 64 (ScalarE) -> partition 0 (DMA) -> reciprocal at
                # base 0 (custom DVE op misbehaves at base 64). The PE
                # broadcast + normalize are deferred past the head loop so the
                # PE never stalls on this chain.
                nc.vector.tensor_copy(An[:, h, :], op[0:64, :])
                dnm = Ppr.tile([65, 512], FP32, tag="dnm", bufs=4,
                               name=f"dnm_{tag}{h}")
                nc.scalar.activation(dnm[64:65, :], op[64:65, :], AF.Copy)
                dma(dnm[0:1, :], dnm[64:65, :])
                rc = Ppr.tile([1, 512], FP32, tag="rc", bufs=4,
                              name=f"rc_{tag}{h}")
                nc.vector.reciprocal_approx_fast(rc[0:1, :], dnm[0:1, :])
                recs.append(rc)
            for h in range(H):
                bc = Pout.tile([65, 512], FP32, tag="out", name=f"bc_{tag}{h}")
                nc.tensor.matmul(bc[0:64, :], ones[0:1, 0:64], recs[h][0:1, :],
                                 start=True, stop=True)
                nc.vector.tensor_mul(An[:, h, :], An[:, h, :], bc[0:64, :])
            # merge: accumulate per head (K=64), + bm_eff at evacuation
            mg = Psc.tile([128, 2, 512], FP32, tag="sc", name=f"mg_{tag}")
            for og in range(2):
                for h in range(H):
                    nc.tensor.matmul(mg[:, og, :],
                                     WB[0:64, WWM + h * 256 + og * 128:
                                        WWM + h * 256 + (og + 1) * 128],
                                     An[:, h, :],
                                     start=(h == 0), stop=(h == 3))
            for og in range(2):
                nc.scalar.activation(merge_b16[:, og, :], mg[:, og, :],
                                     AF.Identity, bias=bb(2 + og))

        # K/V for rounds 1a/1b depend only on the own scaled shards: project
        # + launch their AllGathers back to back so both fly during the
        # round-1a stream
        ags_r1a = kv_own("r1a", xt_own_bf, xt_own_bf)
        ags_r1b = kv_own("r1b", xs_own_bf, xs_own_bf)
        coll("AllReduce", ALU.add, ar_in, ar_out, D)

        # ---------------- round 1a (dis AllReduce completes underneath) ----------
        attn_core("r1a", ags_r1a, xs_own_bf, d_bf)

        # ---- dis BN apply + relu + d2 (own shard) ----
        dma(tot[:, :, :], ar_out[:, :].rearrange("(g p) c -> p g c", g=2))

        def bn_coefs(ng, tot_t, ci, g_cols, be_cols):
            """Vectorized BN coefficients for ng row-groups at once:
            cfw[:, 2, :ng]=scale a, cfw[:, 3, :ng]=bias b."""
            mu, va, a_, b_ = (cfw[:, k, 0:ng] for k in range(4))
            t_, r_, s1, s2 = (cfw[:, 4 + k, 0:ng] for k in range(4))
            nc.vector.tensor_scalar_mul(mu, tot_t[:, :, ci], 1.0 / N)
            nc.vector.tensor_scalar_mul(va, tot_t[:, :, ci + 1], 1.0 / N)
            nc.vector.tensor_mul(s1, mu, mu)
            nc.vector.tensor_sub(va, va, s1)
            rsqrt_into(t_, r_, s1, s2, va, EPS)
            nc.vector.tensor_mul(a_, g_cols, r_)
            nc.vector.tensor_mul(b_, mu, a_)
            nc.vector.tensor_scalar_mul(b_, b_, -1.0)
            nc.vector.tensor_add(b_, b_, be_cols)

        for i, (x1, dst) in enumerate(((x1d_s, ds_own_bf), (x1d_t, dt_own_bf))):
            bn_coefs(2, tot, 2 * i, BB[:, 6:8], BB[:, 8:10])
            for og in range(2):
                nc.scalar.activation(r_d[:, og, :], x1[:, og, :], AF.Relu,
                                     bias=cfw[:, 3, og:og + 1],
                                     scale=cfw[:, 2, og:og + 1])
            for og in range(2):
                mp = Psc.tile([128, 2, 512], FP32, tag="sc", name="mp_d2")
                for cg in range(2):
                    nc.tensor.matmul(mp[:, 0, :], wsl(WD2, cg, og),
                                     r_d[:, cg, :], start=(cg == 0), stop=(cg == 1))
                nc.vector.tensor_scalar_add(dst[:, og, :], mp[:, 0, :],
                                            bb(10 + og))
        # round 2a K/V: keys from dt (own shard), values = (delta0*xt) own
        # shard - gate in place, project, AllGather (flies during round 1b)
        for g in range(2):
            nc.vector.tensor_mul(d_bf[:, g, :], d_bf[:, g, :],
                                 xt_own_bf[:, g, :])
        ags_r2a = kv_own("r2a", dt_own_bf, d_bf)

        # ---------------- round 1b --------
        attn_core("r1b", ags_r1b, xt_own_bf, d1own_bf)

        # round 2b K/V: keys from ds, values = (delta1*xs) own shard
        for g in range(2):
            nc.vector.tensor_mul(d1own_bf[:, g, :], d1own_bf[:, g, :],
                                 xs_own_bf[:, g, :])
        ags_r2b = kv_own("r2b", ds_own_bf, d1own_bf)

        # ---------------- round 2 ----
        attn_core("r2a", ags_r2a, ds_own_bf, dp0_bf)
        attn_core("r2b", ags_r2b, dt_own_bf, dp1_bf)

        # ---------------- final mlp (sharded, means only) + MMD ----------------
        x1_s = PB.tile([128, 4, 512], FP32, name="x1_s", tag="x1_s")
        x1_t = PB.tile([128, 4, 512], FP32, name="x1_t", tag="Kb")
        sq = PB.tile([128, 4, 512], FP32, name="sq", tag="VT")
        for i, (xo, dp, x1) in enumerate(((xs_own_bf, dp0_bf, x1_s),
                                          (xt_own_bf, dp1_bf, x1_t))):
            rhs = [xo[:, 0, :], xo[:, 1, :], dp[:, 0, :], dp[:, 1, :]]
            for og in range(4):
                mp = Psc.tile([128, 2, 512], FP32, tag="sc", name="mp_m1")
                for cg in range(4):
                    nc.tensor.matmul(mp[:, 0, :], wsl(WM1, cg, og, cgw=512),
                                     rhs[cg], start=(cg == 0), stop=(cg == 3))
                nc.vector.tensor_scalar_add(x1[:, og, :], mp[:, 0, :],
                                            bb(12 + og))
            nc.vector.reduce_sum(parf[:, :, 2 * i], x1[:, :, :], axis=AX.X)
            nc.vector.tensor_mul(sq[:, :, :], x1[:, :, :], x1[:, :, :])
            nc.vector.reduce_sum(parf[:, :, 2 * i + 1], sq[:, :, :], axis=AX.X)
        arf_in = Dram.tile([2 * D, 4], FP32, name="arf_in")
        arf_out = Dram.tile([2 * D, 4], FP32, name="arf_out", addr_space=shr)
        dma(arf_in[:, :].rearrange("(g p) c -> p g c", g=4), parf[:, :, :])
        coll("AllReduce", ALU.add, arf_in, arf_out, 2 * D)
        dma(totf[:, :, :], arf_out[:, :].rearrange("(g p) c -> p g c", g=4))

        r_f = PB.tile([128, 4, 512], FP32, name="r_f", tag="VT")
        for i, x1 in enumerate((x1_s, x1_t)):
            bn_coefs(4, totf, 2 * i, BB[:, 16:20], BB[:, 20:24])
            for og in range(4):
                nc.scalar.activation(r_f[:, og, :], x1[:, og, :], AF.Relu,
                                     bias=cfw[:, 3, og:og + 1],
                                     scale=cfw[:, 2, og:og + 1])
            nc.vector.reduce_sum(mrel[:, :, i], r_f[:, :, :], axis=AX.X)
        ars_in = Dram.tile([2 * D, 2], FP32, name="ars_in")
        ars_out = Dram.tile([2 * D, 2], FP32, name="ars_out", addr_space=shr)
        dma(ars_in[:, :].rearrange("(g p) c -> p g c", g=4), mrel[:, :, :])
        coll("AllReduce", ALU.add, ars_in, ars_out, 2 * D)
        dma(totr[:, :, :], ars_out[:, :].rearrange("(g p) c -> p g c", g=4))

        # delta = m2 @ (mean relu_s - mean relu_t): fp32 matvec (N=1)
        nc.vector.tensor_sub(mdif[:, :, :], totr[:, :, 0:1], totr[:, :, 1:2])
        nc.vector.tensor_scalar_mul(mdif[:, :, :], mdif[:, :, :], 1.0 / N)
        mv = Psc.tile([128, 2, 512], FP32, tag="sc", name="mv")
        for og in range(2):
            for cg in range(4):
                c0 = cg * 256 + og * 128
                nc.tensor.matmul(mv[:, og, 0:1], M2F[:, c0:c0 + 128],
                                 mdif[:, cg, :], start=(cg == 0), stop=(cg == 3))
        for og in range(2):
            nc.vector.tensor_copy(dlt[:, og, :], mv[:, og, 0:1])
        nc.vector.tensor_mul(dsq[:, :, :], dlt[:, :, :], dlt[:, :, :])
        dot = Pout.tile([65, 512], FP32, tag="out", name="dot")
        for g in range(2):
            nc.tensor.matmul(dot[0:1, 0:1], dsq[:, g, :], ones[:, 0:1],
                             start=(g == 0), stop=(g == 1))
        nc.vector.tensor_copy(res[:, :], dot[0:1, 0:1])
        dma(out_dram[:, :], res[:, :])

        st.close()

    nc.compile()
    return nc


# head permutation: new row i = h*64+d  <- old channel d*4+h
_PERM = np.array([d * H + h for h in range(H) for d in range(HD)])


def _prep_inputs(inputs):
    bf16 = ml_dtypes.bfloat16
    f32 = np.float32

    def C(x, dt=f32):
        return np.ascontiguousarray(np.asarray(x), dtype=dt)

    def pack2(w):  # [256, X] -> [128, 2*X] (row-group major)
        return w.reshape(2, 128, w.shape[1]).transpose(1, 0, 2).reshape(128, -1)

    p_src = C(inputs["p_src"])[0]
    p_tar = C(inputs["p_tar"])[0]
    dis_src = C(inputs["dis_src"])[0]
    dis_tar = C(inputs["dis_tar"])[0]
    aq_w = C(inputs["aq_w"]); ak_w = C(inputs["ak_w"])
    av_w = C(inputs["av_w"]); am_w = C(inputs["am_w"])
    wqT = aq_w[_PERM, :].T.copy()
    wkT = ak_w[_PERM, :].T.copy()
    wvT = av_w[_PERM, :].T.copy()
    wmT = am_w[:, _PERM].T.copy()
    d2T = C(inputs["d2_w"]).T.copy()
    d1T = C(inputs["d1_w"]).T.copy()
    m1T = C(inputs["m1_w"]).T.copy()
    m2T = C(inputs["m2_w"]).T.copy()
    wm_pack = np.concatenate(
        [wmT.reshape(4, 64, 256).transpose(1, 0, 2).reshape(64, 1024),
         np.zeros((64, 1024), f32)], axis=0)
    wblob = np.concatenate([
        pack2(wqT), pack2(wkT), pack2(wvT), pack2(d2T), d1T,
        m1T.reshape(4, 128, 512).transpose(1, 0, 2).reshape(128, 2048),
        wm_pack,
    ], axis=1)
    assert wblob.shape == (128, NWB)
    m2f = m2T.reshape(4, 128, 256).transpose(1, 0, 2).reshape(128, 1024)

    def col2(v):  # [256] -> [128, 2]
        return C(v).reshape(2, 128).T.copy()

    def col4(v):  # [512] -> [128, 4]
        return C(v).reshape(4, 128).T.copy()

    bm_eff = C(inputs["am_b"]) + am_w @ C(inputs["av_b"])
    bblob = np.concatenate([
        col2(C(inputs["aq_b"])[_PERM]),
        col2(bm_eff),
        col2(inputs["d1_b"]), col2(inputs["d1_g"]), col2(inputs["d1_be"]),
        col2(inputs["d2_b"]),
        col4(inputs["m1_b"]), col4(inputs["m1_g"]), col4(inputs["m1_be"]),
    ], axis=1)
    assert bblob.shape == (128, NBB)

    shared = {
        "psT": C(p_src.T, bf16), "ptT": C(p_tar.T, bf16),
        "dsT": C(dis_src.T, bf16), "dtT": C(dis_tar.T, bf16),
        "wblob": C(wblob, bf16),
        "m2f": C(m2f),
        "bblob": C(bblob),
    }
    in_maps = []
    for c in range(NC):
        sl = slice(c * NQ, (c + 1) * NQ)
        m = dict(shared)
        m["ops"] = C(p_src[sl, :].T, bf16)
        m["opt"] = C(p_tar[sl, :].T, bf16)
        m["ods"] = C(dis_src[sl, :].T, bf16)
        m["odt"] = C(dis_tar[sl, :].T, bf16)
        in_maps.append(m)
    return in_maps


def kernel(**inputs):
    from concourse.bass_utils import run_bass_kernel_spmd

    if "nc" not in _CACHE:
        _CACHE["nc"] = _build_program()
    nc = _CACHE["nc"]
    in_maps = _prep_inputs(inputs)
    res = run_bass_kernel_spmd(nc, in_maps, core_ids=list(range(NC)))
    return np.asarray(res.results[0]["out"], np.float32).reshape(())


# revision 27
# speedup vs baseline: 1.0732x; 1.0732x over previous
"""Trainium2 Bass kernel for nn_AttentionalGNN (self-contained).

  xs/xt = standardize(p_src/p_tar).T ; ds/dt = mlp_dis(standardize(dis).T)
  delta0 = attn(xs, xt, xt); delta1 = attn(xt, xs, xs)
  ps = delta0*xt; pt = delta1*xs
  delta0' = attn(ds, dt, ps); delta1' = attn(dt, ds, pt)
  out_s = xs + mlp(cat(xs, delta0')); out_t likewise
  return ||mean_n(out_s) - mean_n(out_t)||^2

8-core SPMD: scale stats replicated; queries sharded 512/core for all four
attention calls (keys/values replicated); mlp_dis sharded over N with BN-stat
AllReduce + AllGather; the two round-1 deltas are AllGathered separately
(delta0 right after round 1a so the wire time hides under round 1b; delta1
after 1b, hiding under round 2a); final MLP sharded over N with AllReduced BN
stats.

Since standardize() gives every feature column exactly zero mean, the
residual xs/xt terms vanish from the final MMD: the scalar reduces to
||m2 @ (mean_n relu_s - mean_n relu_t)||^2, so m2 is applied to the
512-vector of relu means (fp32) instead of the full N columns.

Attention uses transposed scores (keys on partitions, queries on free) so no
transposes are needed anywhere: scoresT = K_h^T Q_h via one K=64 matmul per
key m-tile; exp on ScalarE (scale=1/8, no max subtraction - scores are O(10));
softmax denominator comes from a ones column prepended to V^T inside the PV
matmul; per-head normalization happens post-loop with a PE-broadcast
reciprocal. K-projection bias is dropped entirely: it adds a per-query
constant to every key's score, which softmax cancels. V bias is folded into
the merge bias (bm_eff = am_b + am_w @ av_b). Head channels are permuted
host-side (d*4+h -> h*64+d) so head slices are contiguous.

All bf16 weights ship as one packed [128, 5376] blob (single DMA on the ACT
HWDGE queue so it never queues behind the big input loads on SP), biases as
one [128, 24] fp32 blob. 1/sqrt is computed on VectorE with the bit-trick +
2 Newton steps so ScalarE never swaps activation tables away from exp.
"""

import numpy as np
import ml_dtypes

D, H, HD, S, N, EPS = 256, 4, 64, 128, 4096, 1e-5
NC = 8
NQ = N // NC            # 512 queries per core
MT = N // 128           # 32 key m-tiles
HB = HD + 1             # per-head V^T block: [ones | V] = 65 cols

# bf16 weight blob column offsets
WQ0, WK0, WV0, WD2, WD1, WM1, WWM = 0, 512, 1024, 1536, 2048, 2304, 4352
NWB = 5376
# fp32 bias blob columns: bq0:2 bm2:4 d1b4:6 d1g6:8 d1be8:10 d2b10:12
# m1b12:16 m1g16:20 m1be20:24
NBB = 24

_CACHE = {}


def _build_program(single=False):
    """single=True: replace collectives with same-size local DMA copies so the
    program is single-core simulatable - timing proxy only."""
    import contextlib
    import concourse.bass as bass
    import concourse.bacc as bacc
    import concourse.tile as tile
    import concourse.mybir as mybir

    FP32 = mybir.dt.float32
    BF16 = mybir.dt.bfloat16
    I32 = mybir.dt.int32
    AF = mybir.ActivationFunctionType
    ALU = mybir.AluOpType
    AX = mybir.AxisListType

    nc = bacc.Bacc(
        "TRN2",
        target_bir_lowering=False,
        debug=False,
        enable_asserts=False,
        num_devices=NC,
    )

    def din(name, shape, dt):
        return nc.dram_tensor(name, shape, dt, kind="ExternalInput").ap()

    psT = din("psT", [D, N], BF16)
    ptT = din("ptT", [D, N], BF16)
    dsT = din("dsT", [S, N], BF16)
    dtT = din("dtT", [S, N], BF16)
    ops = din("ops", [D, NQ], BF16)
    opt_ = din("opt", [D, NQ], BF16)
    ods = din("ods", [S, NQ], BF16)
    odt = din("odt", [S, NQ], BF16)
    wblob = din("wblob", [128, NWB], BF16)
    m2f = din("m2f", [128, 4 * D], FP32)
    bblob = din("bblob", [128, NBB], FP32)
    out_dram = nc.dram_tensor("out", [1, 1], FP32, kind="ExternalOutput").ap()

    RG = [list(range(NC))]

    with tile.TileContext(nc) as tc:
        st = contextlib.ExitStack()
        PA = st.enter_context(tc.tile_pool(name="persistA", bufs=1))
        Ppr = st.enter_context(tc.tile_pool(name="probs", bufs=4))
        Psc = st.enter_context(
            tc.tile_pool(name="psum_sc", bufs=3, space=bass.MemorySpace.PSUM))
        Pout = st.enter_context(
            tc.tile_pool(name="psum_out", bufs=2, space=bass.MemorySpace.PSUM))
        Dram = st.enter_context(tc.tile_pool(name="dram", bufs=1, space="DRAM"))

        def pa(name, shape, dt, tag=None):
            return PA.tile(shape, dt, name=name, tag=tag or name)

        # --- persistA: needed from preprocessing onward ---
        WB = pa("WB", [128, NWB], BF16)
        M2F = pa("M2F", [128, 4 * D], FP32)
        BB = pa("BB", [128, NBB], FP32)
        xs_bf = pa("xs_bf", [128, 2, 8, 512], BF16)
        xt_bf = pa("xt_bf", [128, 2, 8, 512], BF16)
        xs_own_bf = pa("xs_own_bf", [128, 2, 512], BF16)
        xt_own_bf = pa("xt_own_bf", [128, 2, 512], BF16)
        ds_own_bf = pa("ds_own_bf", [128, 2, 512], BF16)
        dt_own_bf = pa("dt_own_bf", [128, 2, 512], BF16)
        x1d_s = pa("x1d_s", [128, 2, 512], FP32)
        x1d_t = pa("x1d_t", [128, 2, 512], FP32)
        sc_ds = pa("sc_ds", [128, 512], BF16)
        sc_dt = pa("sc_dt", [128, 512], BF16)
        r_d = pa("r_d", [128, 2, 512], BF16)
        Qb = pa("Qb", [128, 2, 512], BF16)
        An = pa("An", [64, 4, 512], BF16)       # attn out per head (raw->normed)
        d_bf = pa("d_bf", [128, 2, 512], BF16)  # round-1 delta0 own
        d1own_bf = pa("d1own_bf", [128, 2, 512], BF16)
        dp0_bf = pa("dp0_bf", [128, 2, 512], BF16)
        dp1_bf = pa("dp1_bf", [128, 2, 512], BF16)
        ones = pa("ones", [128, 64], FP32)
        sqd = pa("sqd", [128, 2, 512], FP32)
        bns = pa("bns", [128, 8, 6], FP32)
        par = pa("par", [128, 2, 4], FP32)
        tot = pa("tot", [128, 2, 4], FP32)
        parf = pa("parf", [128, 4, 4], FP32)
        totf = pa("totf", [128, 4, 4], FP32)
        mrel = pa("mrel", [128, 4, 2], FP32)
        totr = pa("totr", [128, 4, 2], FP32)
        mdif = pa("mdif", [128, 4, 1], FP32)
        cfw = pa("cfw", [128, 8, 4], FP32)   # vectorized BN coef scratch
        dlt = pa("dlt", [128, 2, 1], FP32)
        dsq = pa("dsq", [128, 2, 1], FP32)
        res = pa("res", [1, 1], FP32)
        sv = pa("sv", [128, 200], FP32)   # scalar scratch, allocator below

        _svc = [0]

        def scol(n=1):
            b = _svc[0]
            _svc[0] += n
            assert _svc[0] <= 200
            return [sv[:, b + i:b + i + 1] for i in range(n)]

        dma = nc.sync.dma_start
        shr = "Local" if single else "Shared"

        def coll(kind, op, in_t, out_t, in_rows):
            if not single:
                nc.gpsimd.collective_compute(kind, op, replica_groups=RG,
                                             ins=[in_t.opt()],
                                             outs=[out_t.opt()])
            elif kind == "AllGather":
                for r_ in range(NC):
                    dma(out_t[r_ * in_rows:(r_ + 1) * in_rows, :], in_t[:, :])
            else:
                dma(out_t[:, :], in_t[:, :])

        # weight slice helpers (og in units of 128 output cols)
        def wsl(base, cg, og, cgw=256):
            c0 = base + cg * cgw + og * 128
            return WB[:, c0:c0 + 128]

        def bb(col):
            return BB[:, col:col + 1]

        def rsqrt_into(t, r, s1, s2, var_ap, eps):
            """1/sqrt(var+eps) -> r on VectorE: bit-trick seed + 2 Newton.
            t/r/s1/s2 are same-shape fp32 scratch APs."""
            nc.vector.tensor_scalar_add(t, var_ap, float(eps))
            ti = t.bitcast(I32)
            ri = r.bitcast(I32)
            nc.vector.tensor_scalar(ri, ti, 1, None,
                                    ALU.logical_shift_right)
            nc.vector.tensor_scalar(ri, ri, 0x5F3759DF, -1,
                                    ALU.subtract, ALU.mult)
            for _ in range(2):
                nc.vector.tensor_mul(s1, r, r)
                nc.vector.tensor_mul(s1, s1, t)
                nc.vector.tensor_scalar(s2, s1, -0.5, 1.5, ALU.mult, ALU.add)
                nc.vector.tensor_mul(r, r, s2)
            return r

        def rsqrt_dve(var_ap, eps):
            t, r, s1, s2 = scol(4)
            return rsqrt_into(t, r, s1, s2, var_ap, eps)

        # weight blobs ride the ACT HWDGE queue so the big input loads own SP
        nc.scalar.dma_start(WB[:, :], wblob[:, :])
        nc.scalar.dma_start(M2F[:, :], m2f[:, :])
        nc.scalar.dma_start(BB[:, :], bblob[:, :])
        nc.vector.memset(ones[:, :], 1.0)

        # ------------- standardize p_src/p_tar (per row-group stream) -------------
        with tc.tile_pool(name="pin", bufs=3) as Pin:
            for nm, srcT, ownT, dst_bf, own_b16 in (
                ("s", psT, ops, xs_bf, xs_own_bf),
                ("t", ptT, opt_, xt_bf, xt_own_bf),
            ):
                for g in range(2):
                    pbuf = Pin.tile([128, 8, 512], BF16, tag="pbuf",
                                    name=f"pbuf_{nm}{g}")
                    obuf = Pin.tile([128, 512], BF16, tag="obuf",
                                    name=f"obuf_{nm}{g}")
                    r = slice(g * 128, (g + 1) * 128)
                    dma(pbuf[:, :, :], srcT[r, :].rearrange("p (c f) -> p c f", f=512))
                    dma(obuf[:, :], ownT[r, :])
                    for c in range(8):
                        nc.vector.bn_stats(bns[:, c, :], pbuf[:, c, :])
                    ag2 = pa(f"ag_{nm}{g}", [128, 2], FP32)
                    nc.vector.bn_aggr(ag2[:, :], bns[:, :, :])
                    inv = rsqrt_dve(ag2[:, 1:2], 0.0)
                    (nb,) = scol(1)
                    nc.vector.tensor_mul(nb, ag2[:, 0:1], inv)
                    nc.vector.tensor_scalar_mul(nb, nb, -1.0)
                    for hh in range(2):
                        nc.scalar.activation(dst_bf[:, g, 4 * hh:4 * hh + 4, :],
                                             pbuf[:, 4 * hh:4 * hh + 4, :],
                                             AF.Identity, bias=nb, scale=inv)
                    nc.scalar.activation(own_b16[:, g, :], obuf[:, :],
                                         AF.Identity, bias=nb, scale=inv)

            # ------------- dis stats + own shard scale + d1 conv -------------
            for nm, srcT, ownT, scdst in (
                ("ds", dsT, ods, sc_ds),
                ("dt", dtT, odt, sc_dt),
            ):
                dbuf = Pin.tile([128, 8, 512], BF16, tag="pbuf", name=f"dbuf_{nm}")
                obuf = Pin.tile([128, 512], BF16, tag="obuf", name=f"obuf_{nm}")
                dma(dbuf[:, :, :], srcT[:, :].rearrange("p (c f) -> p c f", f=512))
                dma(obuf[:, :], ownT[:, :])
                for c in range(8):
                    nc.vector.bn_stats(bns[:, c, :], dbuf[:, c, :])
                ag2 = pa(f"ag_{nm}", [128, 2], FP32)
                nc.vector.bn_aggr(ag2[:, :], bns[:, :, :])
                inv = rsqrt_dve(ag2[:, 1:2], 0.0)
                (nb,) = scol(1)
                nc.vector.tensor_mul(nb, ag2[:, 0:1], inv)
                nc.vector.tensor_scalar_mul(nb, nb, -1.0)
                nc.scalar.activation(scdst[:, :], obuf[:, :],
                                     AF.Identity, bias=nb, scale=inv)

        for src, dst in ((sc_ds, x1d_s), (sc_dt, x1d_t)):
            for og in range(2):
                mp = Psc.tile([128, 2, 512], FP32, tag="sc", name="mp_d1")
                nc.tensor.matmul(mp[:, 0, :], WB[:, WD1 + og * 128:WD1 + (og + 1) * 128],
                                 src[:, :], start=True, stop=True)
                nc.vector.tensor_scalar_add(dst[:, og, :], mp[:, 0, :],
                                            bb(4 + og))
        # partial BN stats for both d1 outputs -> one AllReduce
        for i, x1 in enumerate((x1d_s, x1d_t)):
            nc.vector.reduce_sum(par[:, :, 2 * i], x1[:, :, :], axis=AX.X)
            nc.vector.tensor_mul(sqd[:, :, :], x1[:, :, :], x1[:, :, :])
            nc.vector.reduce_sum(par[:, :, 2 * i + 1], sqd[:, :, :], axis=AX.X)
        ar_in = Dram.tile([D, 4], FP32, name="ar_in")
        ar_out = Dram.tile([D, 4], FP32, name="ar_out", addr_space=shr)
        dma(ar_in[:, :].rearrange("(g p) c -> p g c", g=2), par[:, :, :])
        coll("AllReduce", ALU.add, ar_in, ar_out, D)

        # --- persistB: attention-era tensors ---
        PB = st.enter_context(tc.tile_pool(name="persistB", bufs=1))

        VT = PB.tile([128, MT, H * HB], BF16, name="VT", tag="VT")
        Kb = PB.tile([128, 2, 8, 512], BF16, name="Kb", tag="Kb")
        ds_bf = PB.tile([128, 2, 8, 512], BF16, name="ds_bf", tag="ds_bf")
        dt_bf = PB.tile([128, 2, 8, 512], BF16, name="dt_bf", tag="dt_bf")
        d0f = PB.tile([128, 2, 8, 512], BF16, name="d0f", tag="d0f")
        for h in range(H):
            nc.vector.memset(VT[:, :, h * HB + HD], 1.0)

        def attention(tag, q_own_bf, k_src, v_src, merge_b16):
            # Q projection (+bias)
            qp = Psc.tile([128, 2, 512], FP32, tag="sc", name=f"qp_{tag}")
            for og in range(2):
                for cg in range(2):
                    nc.tensor.matmul(qp[:, og, :], wsl(WQ0, cg, og),
                                     q_own_bf[:, cg, :],
                                     start=(cg == 0), stop=(cg == 1))
            for og in range(2):
                nc.scalar.activation(Qb[:, og, :], qp[:, og, :], AF.Identity,
                                     bias=bb(0 + og))
            # K projection, full N (no bias: softmax cancels per-query consts)
            for og in range(2):
                for c in range(8):
                    kp = Psc.tile([128, 2, 512], FP32, tag="sc", name=f"kp_{tag}")
                    for cg in range(2):
                        nc.tensor.matmul(kp[:, 0, :], wsl(WK0, cg, og),
                                         k_src[:, cg, c, :],
                                         start=(cg == 0), stop=(cg == 1))
                    nc.vector.tensor_copy(Kb[:, og, c, :], kp[:, 0, :])
            # V^T projection (keys on partitions), no bias (folded into bm)
            for m in range(MT):
                c, f0 = divmod(m * 128, 512)
                vp = Psc.tile([128, 2, 512], FP32, tag="sc", name=f"vp_{tag}")
                for cg in range(2):
                    nc.tensor.matmul(vp[:, 0, 0:256],
                                     v_src[:, cg, c, f0:f0 + 128],
                                     WB[:, WV0 + cg * 256:WV0 + (cg + 1) * 256],
                                     start=(cg == 0), stop=(cg == 1))
                nc.vector.tensor_copy(
                    VT[:, m, :].rearrange("p (h c) -> p h c", c=HB)[:, :, 0:HD],
                    vp[:, 0, 0:256].rearrange("p (h c) -> p h c", c=HD))
            # streaming attention per head, PV pipelined one group behind
            recs = []
            for h in range(H):
                hg, hp = h // 2, (h % 2) * 64
                op = Pout.tile([65, 512], FP32, tag="out", name=f"op_{tag}{h}")
                prev = None
                for g in range(16):
                    sc = Psc.tile([128, 2, 512], FP32, tag="sc", name=f"sc_{tag}")
                    for j in range(2):
                        m = g * 2 + j
                        c, f0 = divmod(m * 128, 512)
                        nc.tensor.matmul(sc[:, j, :],
                                         Kb[hp:hp + 64, hg, c, f0:f0 + 128],
                                         Qb[hp:hp + 64, hg, :],
                                         start=True, stop=True)
                    pr = Ppr.tile([128, 2, 512], BF16, tag="pr", name=f"pr_{tag}",
                                  bufs=3)
                    nc.scalar.activation(pr[:, :, :], sc[:, :, :], AF.Exp,
                                         scale=0.125)
                    if prev is not None:
                        for j in range(2):
                            m = prev[0] * 2 + j
                            nc.tensor.matmul(
                                op[:, :], VT[:, m, h * HB:(h + 1) * HB],
                                prev[1][:, j, :], start=(m == 0), stop=False)
                    prev = (g, pr)
                for j in range(2):
                    m = prev[0] * 2 + j
                    nc.tensor.matmul(op[:, :], VT[:, m, h * HB:(h + 1) * HB],
                                     prev[1][:, j, :], start=False,
                                     stop=(m == MT - 1))
                # evacuate raw numerator (bf16); denominator: psum row 64 ->
                # sbuf row 64 (ScalarE) -> partition 0 (DMA) -> reciprocal at
                # base 0 (custom DVE op misbehaves at base 64). The PE
                # broadcast + normalize are deferred past the head loop so the
                # PE never stalls on this chain.
                nc.vector.tensor_copy(An[:, h, :], op[0:64, :])
                dnm = Ppr.tile([65, 512], FP32, tag="dnm", bufs=4,
                               name=f"dnm_{tag}{h}")
                nc.scalar.activation(dnm[64:65, :], op[64:65, :], AF.Copy)
                dma(dnm[0:1, :], dnm[64:65, :])
                rc = Ppr.tile([1, 512], FP32, tag="rc", bufs=4,
                              name=f"rc_{tag}{h}")
                nc.vector.reciprocal_approx_fast(rc[0:1, :], dnm[0:1, :])
                recs.append(rc)
            for h in range(H):
                bc = Pout.tile([65, 512], FP32, tag="out", name=f"bc_{tag}{h}")
                nc.tensor.matmul(bc[0:64, :], ones[0:1, 0:64], recs[h][0:1, :],
                                 start=True, stop=True)
                nc.vector.tensor_mul(An[:, h, :], An[:, h, :], bc[0:64, :])
            # merge: accumulate per head (K=64), + bm_eff at evacuation
            mg = Psc.tile([128, 2, 512], FP32, tag="sc", name=f"mg_{tag}")
            for og in range(2):
                for h in range(H):
                    nc.tensor.matmul(mg[:, og, :],
                                     WB[0:64, WWM + h * 256 + og * 128:
                                        WWM + h * 256 + (og + 1) * 128],
                                     An[:, h, :],
                                     start=(h == 0), stop=(h == 3))
            for og in range(2):
                nc.scalar.activation(merge_b16[:, og, :], mg[:, og, :],
                                     AF.Identity, bias=bb(2 + og))

        # ---------------- round 1a (dis AllReduce completes underneath) ----------
        attention("r1a", xs_own_bf, xt_bf, xt_bf, d_bf)
        ag0_in = Dram.tile([D, NQ], BF16, name="ag0_in")
        ag0_out = Dram.tile([NC * D, NQ], BF16, name="ag0_out",
                            addr_space=shr)
        dma(ag0_in[:, :].rearrange("(g p) f -> p g f", g=2), d_bf[:, :, :])
        coll("AllGather", ALU.bypass, ag0_in, ag0_out, D)

        # ---- dis BN apply + relu + d2 (own shard) + AllGather ds/dt ----
        dma(tot[:, :, :], ar_out[:, :].rearrange("(g p) c -> p g c", g=2))

        def bn_coefs(ng, tot_t, ci, g_cols, be_cols):
            """Vectorized BN coefficients for ng row-groups at once:
            cfw[:, 2, :ng]=scale a, cfw[:, 3, :ng]=bias b."""
            mu, va, a_, b_ = (cfw[:, k, 0:ng] for k in range(4))
            t_, r_, s1, s2 = (cfw[:, 4 + k, 0:ng] for k in range(4))
            nc.vector.tensor_scalar_mul(mu, tot_t[:, :, ci], 1.0 / N)
            nc.vector.tensor_scalar_mul(va, tot_t[:, :, ci + 1], 1.0 / N)
            nc.vector.tensor_mul(s1, mu, mu)
            nc.vector.tensor_sub(va, va, s1)
            rsqrt_into(t_, r_, s1, s2, va, EPS)
            nc.vector.tensor_mul(a_, g_cols, r_)
            nc.vector.tensor_mul(b_, mu, a_)
            nc.vector.tensor_scalar_mul(b_, b_, -1.0)
            nc.vector.tensor_add(b_, b_, be_cols)

        for i, (x1, dst) in enumerate(((x1d_s, ds_own_bf), (x1d_t, dt_own_bf))):
            bn_coefs(2, tot, 2 * i, BB[:, 6:8], BB[:, 8:10])
            for og in range(2):
                nc.scalar.activation(r_d[:, og, :], x1[:, og, :], AF.Relu,
                                     bias=cfw[:, 3, og:og + 1],
                                     scale=cfw[:, 2, og:og + 1])
            for og in range(2):
                mp = Psc.tile([128, 2, 512], FP32, tag="sc", name="mp_d2")
                for cg in range(2):
                    nc.tensor.matmul(mp[:, 0, :], wsl(WD2, cg, og),
                                     r_d[:, cg, :], start=(cg == 0), stop=(cg == 1))
                nc.vector.tensor_scalar_add(dst[:, og, :], mp[:, 0, :],
                                            bb(10 + og))
        agd_in = Dram.tile([2 * D, NQ], BF16, name="agd_in")
        agd_out = Dram.tile([NC * 2 * D, NQ], BF16, name="agd_out",
                            addr_space=shr)
        dma(agd_in[0:D, :].rearrange("(g p) f -> p g f", g=2),
            ds_own_bf[:, :, :])
        dma(agd_in[D:2 * D, :].rearrange("(g p) f -> p g f", g=2),
            dt_own_bf[:, :, :])
        coll("AllGather", ALU.bypass, agd_in, agd_out, 2 * D)

        # prefetch gathers: these wait on the two AllGathers and complete
        # while round 1b computes (they sit ahead of r1b's tiny dnm DMAs on
        # SP, which are only consumed post-loop anyway)
        agdv = agd_out[:, :].rearrange("(r h g p) f -> h p g r f", h=2, g=2,
                                       p=128)
        ag0v = ag0_out[:, :].rearrange("(r g p) f -> p g r f", g=2, p=128)
        for g in range(2):
            dma(ds_bf[:, g, :, :], agdv[0][:, g, :, :])
            dma(dt_bf[:, g, :, :], agdv[1][:, g, :, :])
            dma(d0f[:, g, :, :], ag0v[:, g, :, :])

        # ---------------- round 1b (AllGathers complete underneath) --------
        attention("r1b", xt_own_bf, xs_bf, xs_bf, d1own_bf)
        # gate values for round 2a only now: putting these DVE muls before
        # r1b would queue them ahead of r1b's K-evacuation casts and stall
        # its score stream ~27us; here they hide under r2a's Q/K projections
        for g in range(2):
            nc.vector.tensor_mul(d0f[:, g, :, :], d0f[:, g, :, :],
                                 xt_bf[:, g, :, :])
        ag1_in = Dram.tile([D, NQ], BF16, name="ag1_in")
        ag1_out = Dram.tile([NC * D, NQ], BF16, name="ag1_out",
                            addr_space=shr)
        dma(ag1_in[:, :].rearrange("(g p) f -> p g f", g=2), d1own_bf[:, :, :])
        coll("AllGather", ALU.bypass, ag1_in, ag1_out, D)

        # ---------------- round 2a (delta1 AllGather completes underneath) ----
        attention("r2a", ds_own_bf, dt_bf, d0f, dp0_bf)

        # gather delta1 into the xt_bf slot (last read by the d0f gating
        # above) and gate with xs -> pt_tmp
        ag1v = ag1_out[:, :].rearrange("(r g p) f -> p g r f", g=2, p=128)
        for g in range(2):
            dma(xt_bf[:, g, :, :], ag1v[:, g, :, :])
        for g in range(2):
            nc.vector.tensor_mul(xt_bf[:, g, :, :], xt_bf[:, g, :, :],
                                 xs_bf[:, g, :, :])

        attention("r2b", dt_own_bf, ds_bf, xt_bf, dp1_bf)

        # ---------------- final mlp (sharded, means only) + MMD ----------------
        x1_s = PB.tile([128, 4, 512], FP32, name="x1_s", tag="d0f")
        x1_t = PB.tile([128, 4, 512], FP32, name="x1_t", tag="Kb")
        sq = PB.tile([128, 4, 512], FP32, name="sq", tag="VT")
        for i, (xo, dp, x1) in enumerate(((xs_own_bf, dp0_bf, x1_s),
                                          (xt_own_bf, dp1_bf, x1_t))):
            rhs = [xo[:, 0, :], xo[:, 1, :], dp[:, 0, :], dp[:, 1, :]]
            for og in range(4):
                mp = Psc.tile([128, 2, 512], FP32, tag="sc", name="mp_m1")
                for cg in range(4):
                    nc.tensor.matmul(mp[:, 0, :], wsl(WM1, cg, og, cgw=512),
                                     rhs[cg], start=(cg == 0), stop=(cg == 3))
                nc.vector.tensor_scalar_add(x1[:, og, :], mp[:, 0, :],
                                            bb(12 + og))
            nc.vector.reduce_sum(parf[:, :, 2 * i], x1[:, :, :], axis=AX.X)
            nc.vector.tensor_mul(sq[:, :, :], x1[:, :, :], x1[:, :, :])
            nc.vector.reduce_sum(parf[:, :, 2 * i + 1], sq[:, :, :], axis=AX.X)
        arf_in = Dram.tile([2 * D, 4], FP32, name="arf_in")
        arf_out = Dram.tile([2 * D, 4], FP32, name="arf_out", addr_space=shr)
        dma(arf_in[:, :].rearrange("(g p) c -> p g c", g=4), parf[:, :, :])
        coll("AllReduce", ALU.add, arf_in, arf_out, 2 * D)
        dma(totf[:, :, :], arf_out[:, :].rearrange("(g p) c -> p g c", g=4))

        r_f = PB.tile([128, 4, 512], FP32, name="r_f", tag="VT")
        for i, x1 in enumerate((x1_s, x1_t)):
            bn_coefs(4, totf, 2 * i, BB[:, 16:20], BB[:, 20:24])
            for og in range(4):
                nc.scalar.activation(r_f[:, og, :], x1[:, og, :], AF.Relu,
                                     bias=cfw[:, 3, og:og + 1],
                                     scale=cfw[:, 2, og:og + 1])
            nc.vector.reduce_sum(mrel[:, :, i], r_f[:, :, :], axis=AX.X)
        ars_in = Dram.tile([2 * D, 2], FP32, name="ars_in")
        ars_out = Dram.tile([2 * D, 2], FP32, name="ars_out", addr_space=shr)
        dma(ars_in[:, :].rearrange("(g p) c -> p g c", g=4), mrel[:, :, :])
        coll("AllReduce", ALU.add, ars_in, ars_out, 2 * D)
        dma(totr[:, :, :], ars_out[:, :].rearrange("(g p) c -> p g c", g=4))

        # delta = m2 @ (mean relu_s - mean relu_t): fp32 matvec (N=1)
        nc.vector.tensor_sub(mdif[:, :, :], totr[:, :, 0:1], totr[:, :, 1:2])
        nc.vector.tensor_scalar_mul(mdif[:, :, :], mdif[:, :, :], 1.0 / N)
        mv = Psc.tile([128, 2, 512], FP32, tag="sc", name="mv")
        for og in range(2):
            for cg in range(4):
                c0 = cg * 256 + og * 128
                nc.tensor.matmul(mv[:, og, 0:1], M2F[:, c0:c0 + 128],
                                 mdif[:, cg, :], start=(cg == 0), stop=(cg == 3))
        for og in range(2):
            nc.vector.tensor_copy(dlt[:, og, :], mv[:, og, 0:1])
        nc.vector.tensor_mul(dsq[:, :, :], dlt[:, :, :], dlt[:, :, :])
        dot = Pout.tile([65, 512], FP32, tag="out", name="dot")
        for g in range(2):
            nc.tensor.matmul(dot[0:1, 0:1], dsq[:, g, :], ones[:, 0:1],
                             start=(g == 0), stop=(g == 1))
        nc.vector.tensor_copy(res[:, :], dot[0:1, 0:1])
        dma(out_dram[:, :], res[:, :])

        st.close()

    nc.compile()
    return nc


# head permutation: new row i = h*64+d  <- old channel d*4+h
_PERM = np.array([d * H + h for h in range(H) for d in range(HD)])


def _prep_inputs(inputs):
    bf16 = ml_dtypes.bfloat16
    f32 = np.float32

    def C(x, dt=f32):
        return np.ascontiguousarray(np.asarray(x), dtype=dt)

    def pack2(w):  # [256, X] -> [128, 2*X] (row-group major)
        return w.reshape(2, 128, w.shape[1]).transpose(1, 0, 2).reshape(128, -1)

    p_src = C(inputs["p_src"])[0]
    p_tar = C(inputs["p_tar"])[0]
    dis_src = C(inputs["dis_src"])[0]
    dis_tar = C(inputs["dis_tar"])[0]
    aq_w = C(inputs["aq_w"]); ak_w = C(inputs["ak_w"])
    av_w = C(inputs["av_w"]); am_w = C(inputs["am_w"])
    wqT = aq_w[_PERM, :].T.copy()
    wkT = ak_w[_PERM, :].T.copy()
    wvT = av_w[_PERM, :].T.copy()
    wmT = am_w[:, _PERM].T.copy()
    d2T = C(inputs["d2_w"]).T.copy()
    d1T = C(inputs["d1_w"]).T.copy()
    m1T = C(inputs["m1_w"]).T.copy()
    m2T = C(inputs["m2_w"]).T.copy()
    wm_pack = np.concatenate(
        [wmT.reshape(4, 64, 256).transpose(1, 0, 2).reshape(64, 1024),
         np.zeros((64, 1024), f32)], axis=0)
    wblob = np.concatenate([
        pack2(wqT), pack2(wkT), pack2(wvT), pack2(d2T), d1T,
        m1T.reshape(4, 128, 512).transpose(1, 0, 2).reshape(128, 2048),
        wm_pack,
    ], axis=1)
    assert wblob.shape == (128, NWB)
    m2f = m2T.reshape(4, 128, 256).transpose(1, 0, 2).reshape(128, 1024)

    def col2(v):  # [256] -> [128, 2]
        return C(v).reshape(2, 128).T.copy()

    def col4(v):  # [512] -> [128, 4]
        return C(v).reshape(4, 128).T.copy()

    bm_eff = C(inputs["am_b"]) + am_w @ C(inputs["av_b"])
    bblob = np.concatenate([
        col2(C(inputs["aq_b"])[_PERM]),
        col2(bm_eff),
        col2(inputs["d1_b"]), col2(inputs["d1_g"]), col2(inputs["d1_be"]),
        col2(inputs["d2_b"]),
        col4(inputs["m1_b"]), col4(inputs["m1_g"]), col4(inputs["m1_be"]),
    ], axis=1)
    assert bblob.shape == (128, NBB)

    shared = {
        "psT": C(p_src.T, bf16), "ptT": C(p_tar.T, bf16),
        "dsT": C(dis_src.T, bf16), "dtT": C(dis_tar.T, bf16),
        "wblob": C(wblob, bf16),
        "m2f": C(m2f),
        "bblob": C(bblob),
    }
    in_maps = []
    for c in range(NC):
        sl = slice(c * NQ, (c + 1) * NQ)
        m = dict(shared)
        m["ops"] = C(p_src[sl, :].T, bf16)
        m["opt"] = C(p_tar[sl, :].T, bf16)
        m["ods"] = C(dis_src[sl, :].T, bf16)
        m["odt"] = C(dis_tar[sl, :].T, bf16)
        in_maps.append(m)
    return in_maps


def kernel(**inputs):
    from concourse.bass_utils import run_bass_kernel_spmd

    if "nc" not in _CACHE:
        _CACHE["nc"] = _build_program()
    nc = _CACHE["nc"]
    in_maps = _prep_inputs(inputs)
    res = run_bass_kernel_spmd(nc, in_maps, core_ids=list(range(NC)))
    return np.asarray(res.results[0]["out"], np.float32).reshape(())
